# revision 9
# baseline (speedup 1.0000x reference)
"""Causal self-attention (B=2, S=2048, E=1024, H=16, DH=64) on 8 trn2 cores.

Sharding: core c -> (batch b = c//4, head-group g = c%4, heads 4g..4g+3).
Data parallel over batch, tensor parallel over heads, row-sharded Wo;
partial outputs summed on host.

Per-core device kernel (bf16 matmuls, fp32 accumulation):
  phase A: kqT = (x @ Wkq)^T via W-stationary matmuls on xT (+bias on DVE),
           v   =  x @ Wv  (natural layout, [128 sk, 4 heads, 64 dh])
  phase B, per head-pair segment (c, p), per 128-sk tile j:
    st:  scores^T as a 64x128 row-tile PAIR (head A rows 0:64 / head B rows
         64:128) -> two PSUM banks; the pair CO-ISSUES on the PE (measured
         Δstart ~2ns), so both heads cost one N=w stream.
    exp: head A on DVE (Schraudolph bf16 bit-trick), head B on ACT (exact),
         diag tiles masked by a triangular bf16 multiply per head.
    av/rs (deferred 2 tiles for software pipelining): 128x64 col-tile PAIRS:
         av_bank[0:64]  = vA^T P_A   (tile_position (0,0))   } co-issue
         av_bank[64:128]= vB^T P_B   (tile_position (0,64))  }
         rs_bank[0:64]  = ones^T P_A  -> rowsum(A) broadcast to 64 partitions
         rs_bank[64:128]= ones^T P_B  (co-issued pair)
         Attention streams 3*w per tile (st w + av w + rs w) with both heads
         packed, and the softmax denominator arrives pre-broadcast.
    tail (deferred 1 segment): rc = exp(-ln(rs)) on ACT ([128,512] ops - no
         1-partition ops, no gpsimd broadcast), saT = av * rc on DVE.
  phase C: out = saT^T @ Wo in 32 half-tiles [128,512] through the shared
         1-bank PSUM ring, interleaved with the last segment's tail and the
         NEXT rep's projections so the PE never idles across rep boundaries.

PSUM: poolA = 4x 1-bank ring (st pairs, projections, phase C halves),
      poolB = 4x 1-bank ring (av/rs, 2 segments in flight). 8 banks total,
      allocated once - no per-rep pool churn.

kqT/v/saT are double-buffered by rep parity so rep r+1's projections can
overlap rep r's attention/output tail without WAR stalls.
"""
import math

import numpy as np
import ml_dtypes

import concourse.bass as bass
import concourse.bacc as bacc
import concourse.tile as tile
from concourse import mybir
from concourse.masks import make_upper_triangular

BF16 = mybir.dt.bfloat16
F32 = mybir.dt.float32
I16 = mybir.dt.int16
NP_BF16 = ml_dtypes.bfloat16

B, S, E, H, DH = 2, 2048, 1024, 16, 64
N_CORES = 8
HPC = 4          # heads per core
SCH = 4          # number of 512-wide sq chunks
SKT = 16         # number of 128-wide sk tiles
ET = 8           # number of 128-wide e tiles

Exp = mybir.ActivationFunctionType.Exp
Ln = mybir.ActivationFunctionType.Ln
Copy = mybir.ActivationFunctionType.Copy
Mult = mybir.AluOpType.mult
Add = mybir.AluOpType.add

# Schraudolph bf16 exp(s/8): i16 = round(s * SCHRA_A + SCHRA_B), reinterpret
# bits as bf16.  a = 2^7 / (8*ln2); b = 127*2^7 - c with c calibrated to
# center the relative error (max ~ +/-3%, which softmax normalization and
# the AV averaging wash down to ~1e-2 end-to-end).
SCHRA_A = 128.0 / (8.0 * math.log(2.0))
SCHRA_B = 127.0 * 128.0 - 4.6


def build_nc(reps=1):
    nc = bacc.Bacc(None, target_bir_lowering=False)

    xT = nc.dram_tensor("xT", [E, S], BF16, kind="ExternalInput")
    wkq = nc.dram_tensor("wkq", [E, 512], BF16, kind="ExternalInput")
    wv = nc.dram_tensor("wv", [E, 256], BF16, kind="ExternalInput")
    wo = nc.dram_tensor("wo", [256, E], BF16, kind="ExternalInput")
    bkq = nc.dram_tensor("bkq", [128, 4], F32, kind="ExternalInput")
    out = nc.dram_tensor("out", [S, E], F32, kind="ExternalOutput")

    with tile.TileContext(nc) as tc:
        import contextlib
        with contextlib.ExitStack() as ctx:
            const = ctx.enter_context(tc.tile_pool(name="const", bufs=1))
            wpool = ctx.enter_context(tc.tile_pool(name="wpool", bufs=1))
            xpool = ctx.enter_context(tc.tile_pool(name="xpool", bufs=1))
            kqpool = ctx.enter_context(tc.tile_pool(name="kqpool", bufs=1))
            vpool = ctx.enter_context(tc.tile_pool(name="vpool", bufs=1))
            sapool = ctx.enter_context(tc.tile_pool(name="sapool", bufs=1))
            pt_pool = ctx.enter_context(tc.tile_pool(name="pt", bufs=5))
            small = ctx.enter_context(tc.tile_pool(name="small", bufs=3))
            ostage = ctx.enter_context(tc.tile_pool(name="ostage", bufs=4))
            poolA = ctx.enter_context(
                tc.tile_pool(name="pA", bufs=4, space="PSUM"))
            poolB = ctx.enter_context(
                tc.tile_pool(name="pB", bufs=4, space="PSUM"))

            bkq_sb = const.tile([128, 4], F32)
            nc.sync.dma_start(bkq_sb[:], bkq[:])
            triu2 = const.tile([128, 2, 128], BF16)
            make_upper_triangular(nc, triu2[:, 0, :], val=1.0, diag=True)
            make_upper_triangular(nc, triu2[:, 1, :], val=1.0, diag=True)
            ones64 = const.tile([128, 64], BF16, tag="ones64")
            nc.vector.memset(ones64[:], 1.0)

            # interleave weight/activation loads so the first kq matmuls can
            # start as soon as their e-tile arrives
            wkq_sb = wpool.tile([128, ET, 512], BF16)
            xT_sb = xpool.tile([128, ET, S], BF16)
            for e in range(ET):
                nc.sync.dma_start(wkq_sb[:, e, :], wkq[128 * e:128 * (e + 1), :])
                nc.sync.dma_start(xT_sb[:, e, :], xT[128 * e:128 * (e + 1), :])
            wv_sb = wpool.tile([128, ET, 256], BF16)
            nc.sync.dma_start(wv_sb[:], wv.rearrange("(n p) f -> p n f", p=128))
            wo_sb = wpool.tile([128, 2, E], BF16)
            nc.sync.dma_start(wo_sb[:], wo.rearrange("(n p) f -> p n f", p=128))

            # double-buffered by rep parity (cross-rep WAR decoupling)
            kqT_db = [kqpool.tile([128, 4, S], BF16, tag=f"kqT{i}",
                                  name=f"kqT{i}") for i in range(2)]
            v_db = [vpool.tile([128, SKT, HPC, DH], BF16, tag=f"v{i}",
                               name=f"v{i}") for i in range(2)]
            saT_db = [sapool.tile([128, 2, S], BF16, tag=f"saT{i}",
                                  name=f"saT{i}") for i in range(2)]

            def psA():
                return poolA.tile([128, 512], F32, tag="w1", name="w1")

            def psB():
                return poolB.tile([128, 512], F32, tag="w2", name="w2")

            _prev_sa = [None]
            for _rep in range(reps):
                kqT_sb = kqT_db[_rep % 2]
                v_sb = v_db[_rep % 2]
                saT_sb = saT_db[_rep % 2]

                def kq_proj(blk, c):
                    ps = psA()
                    for e in range(ET):
                        nc.tensor.matmul(
                            ps[:], wkq_sb[:, e, 128 * blk:128 * (blk + 1)],
                            xT_sb[:, e, 512 * c:512 * (c + 1)],
                            start=(e == 0), stop=(e == ET - 1))
                    nc.vector.tensor_scalar_add(
                        kqT_sb[:, blk, 512 * c:512 * (c + 1)], ps[:],
                        bkq_sb[:, blk:blk + 1])

                def v_proj(t0, t1):
                    for t in range(t0, t1):
                        ps = psA()
                        for e in range(ET):
                            nc.tensor.matmul(
                                ps[:, 0:256],
                                xT_sb[:, e, 128 * t:128 * (t + 1)],
                                wv_sb[:, e, :],
                                start=(e == 0), stop=(e == ET - 1))
                        nc.scalar.copy(
                            v_sb[:, t, :, :],
                            ps[:, 0:256].rearrange("p (h d) -> p h d", h=HPC))

                av_q = []

                def flush_av(n=0):
                    # Emit deferred av/rs groups (software pipelining: the
                    # PE queue is strict FIFO, so av(j) - which waits on
                    # exp(j) - must queue BEHIND st(j+1)/st(j+2) or the PE
                    # head-blocks). Each group is 4 matmuls in 128x64 mode:
                    # av col-pair + rowsum col-pair, each pair co-issuing.
                    while len(av_q) > n:
                        avb, rsb, pt, p, j, off, first, last = av_q.pop(0)
                        nc.tensor.matmul(avb[0:64, off:512],
                                         v_sb[:, j, 2 * p, :],
                                         pt[:, 0, off:512], start=first,
                                         stop=last, tile_position=(0, 0))
                        nc.tensor.matmul(avb[64:128, off:512],
                                         v_sb[:, j, 2 * p + 1, :],
                                         pt[:, 1, off:512], start=first,
                                         stop=last, tile_position=(0, 64))
                        nc.tensor.matmul(rsb[0:64, off:512], ones64[:],
                                         pt[:, 0, off:512], start=first,
                                         stop=last, tile_position=(0, 0))
                        nc.tensor.matmul(rsb[64:128, off:512], ones64[:],
                                         pt[:, 1, off:512], start=first,
                                         stop=last, tile_position=(0, 64))

                def attn_tiles(c, p):
                    """One head-pair segment: st pair per tile (64x128 mode,
                    co-issued), exp on DVE+ACT, av/rs groups deferred 2 tiles
                    to batch mode switches and hide exp latency."""
                    sq0 = 512 * c
                    kblk, qblk = 2 * p, 2 * p + 1
                    avb, rsb = psB(), psB()
                    nj = 4 * c + 4
                    for j in range(nj):
                        r = j - 4 * c
                        diag = r >= 0
                        off = 128 * r if diag else 0
                        w = 512 - off
                        st0, st1 = psA(), psA()
                        nc.tensor.matmul(
                            st0[:, 0:w],
                            kqT_sb[0:64, kblk, 128 * j:128 * (j + 1)],
                            kqT_sb[0:64, qblk, sq0 + off:sq0 + 512],
                            start=True, stop=True, tile_position=(0, 0))
                        nc.tensor.matmul(
                            st1[:, 0:w],
                            kqT_sb[64:128, kblk, 128 * j:128 * (j + 1)],
                            kqT_sb[64:128, qblk, sq0 + off:sq0 + 512],
                            start=True, stop=True, tile_position=(64, 0))
                        pt = pt_pool.tile([128, 2, 512], BF16, tag="pt")
                        nc.vector.tensor_scalar(
                            pt[:, 0, off:512].bitcast(I16),
                            st0[:, 0:w], SCHRA_A, SCHRA_B, Mult, Add)
                        nc.scalar.activation(pt[:, 1, off:512], st1[:, 0:w],
                                             Exp, scale=0.125)
                        if diag:
                            # causal mask on the (otherwise idle) GPSIMD --
                            # keeps the DVE free for the Schraudolph exps
                            nc.gpsimd.tensor_mul(
                                pt[:, :, off:off + 128],
                                pt[:, :, off:off + 128], triu2[:])
                        av_q.append((avb, rsb, pt, p, j, off, j == 0,
                                     j == nj - 1))
                        flush_av(2)
                    return avb, rsb

                def attn_tail(c, p, avb, rsb):
                    # rc = exp(-ln(rowsum)) on ACT, full-width [128, 512]
                    # (rowsums arrive pre-broadcast from the rs matmuls);
                    # saT = av * rc fused in the PSUM eviction on DVE.
                    sq0 = 512 * c
                    lt = small.tile([128, 512], F32, tag="lt")
                    nc.scalar.activation(lt[:], rsb[:], Ln)
                    rc = small.tile([128, 512], F32, tag="rc")
                    nc.scalar.activation(rc[:], lt[:], Exp, scale=-1.0)
                    nc.vector.tensor_mul(saT_sb[:, p, sq0:sq0 + 512],
                                         avb[:], rc[:])

                def phase_c(sa, t0, t1):
                    # out = sa^T @ Wo in [128, 512] half-tiles through poolA
                    for t in range(t0, t1):
                        for n in range(2):
                            ps = psA()
                            nc.tensor.matmul(
                                ps[:], sa[:, 0, 128 * t:128 * (t + 1)],
                                wo_sb[:, 0, 512 * n:512 * (n + 1)],
                                start=True, stop=False)
                            nc.tensor.matmul(
                                ps[:], sa[:, 1, 128 * t:128 * (t + 1)],
                                wo_sb[:, 1, 512 * n:512 * (n + 1)],
                                start=False, stop=True)
                            ot = ostage.tile([128, 512], F32, tag="ot")
                            if (2 * t + n) % 2 == 0:
                                nc.vector.tensor_copy(ot[:], ps[:])
                            else:
                                nc.scalar.copy(ot[:], ps[:])
                            nc.sync.dma_start(
                                out[128 * t:128 * (t + 1),
                                    512 * n:512 * (n + 1)], ot[:])

                # upfront: only what segment (0,*) needs; the rest of the
                # projections are emitted between segments as PE filler
                for blk in range(4):
                    kq_proj(blk, 0)
                v_proj(0, 4)
                # the previous rep's last phase C chunk (t=12..15 waits on
                # that rep's final tail) is deferred to HERE so its wait
                # hides under this rep's upfront projections
                if _prev_sa[0] is not None:
                    phase_c(_prev_sa[0], 12, 16)
                fillers = {
                    2: lambda: (kq_proj(0, 1), kq_proj(1, 1), v_proj(4, 8)),
                    3: lambda: (kq_proj(2, 1), kq_proj(3, 1)),
                    4: lambda: (kq_proj(0, 2), kq_proj(1, 2), v_proj(8, 12)),
                    5: lambda: (kq_proj(2, 2), kq_proj(3, 2)),
                    6: lambda: (kq_proj(0, 3), kq_proj(1, 3), v_proj(12, 16)),
                    7: lambda: (kq_proj(2, 3), kq_proj(3, 3)),
                }
                pending = None
                for s, (c, p) in enumerate((c, p) for c in range(SCH)
                                           for p in (0, 1)):
                    if s in fillers:
                        fillers[s]()
                    avb, rsb = attn_tiles(c, p)
                    if pending is not None:
                        attn_tail(*pending)
                    pending = (c, p, avb, rsb)
                flush_av(0)
                # phase C interleaved with the last segment's tail: t<12 only
                # needs chunks c<3, whose tails are long done.
                phase_c(saT_sb, 0, 8)
                attn_tail(*pending)
                phase_c(saT_sb, 8, 12)
                _prev_sa[0] = saT_sb
            phase_c_last = _prev_sa[0]
            if phase_c_last is not None:
                phase_c(phase_c_last, 12, 16)

    nc.compile()
    _merge_act_table_loads(nc)
    return nc


def _merge_act_table_loads(nc):
    """Retarget all ACT table loads to natural_log_exp_and_others (holds both
    Exp and Ln) and drop the now-redundant reloads - the default chooser
    alternates exp_and_others/natural_log, reloading tables (~2.7us each)
    at every Exp<->Ln switch."""
    from concourse.hw_specs import get_activation_tables
    tables = get_activation_tables(nc.m.arch)
    combined = list(tables).index("natural_log_exp_and_others")
    fns = tables["natural_log_exp_and_others"]
    for f in nc.m.functions:
        for b in f.blocks:
            first = True
            drop = []
            for ii, ins in enumerate(b.instructions):
                if isinstance(ins, mybir.InstLoadActFuncSet):
                    if first:
                        ins.act_func_set_id = combined
                        first = False
                    else:
                        assert ins.sync_info is None
                        drop.append(ii)
                elif isinstance(ins, mybir.InstActivation):
                    assert ins.func in fns or ins.func in (
                        mybir.ActivationFunctionType.Copy,
                        mybir.ActivationFunctionType.Identity), ins.func
            for ii in reversed(drop):
                del b.instructions[ii]


_CACHE = {}


def _build_runner():
    """Build the SPMD PJRT executable once; returns a dict with a jitted fn.

    Mirrors concourse.bass2jax.run_bass_via_pjrt but hoisted so repeated
    kernel() calls reuse the traced/compiled executable. No donation: the
    kernel DMA-writes every output element, so uninitialized output buffers
    are fine.
    """
    import jax
    from jax.sharding import Mesh, PartitionSpec
    from jax.experimental.shard_map import shard_map
    from concourse import bass2jax as b2j
    from concourse import mybir as _mybir

    if "runner" in _CACHE:
        return _CACHE["runner"]

    nc = _CACHE.get("nc")
    if nc is None:
        nc = _CACHE["nc"] = build_nc()

    b2j.install_neuronx_cc_hook()
    partition_name = (nc.partition_id_tensor.name
                      if nc.partition_id_tensor else None)

    in_names, out_names, out_avals = [], [], []
    for alloc in nc.m.functions[0].allocations:
        if not isinstance(alloc, _mybir.MemoryLocationSet):
            continue
        name = alloc.memorylocations[0].name
        if alloc.kind == "ExternalInput":
            if name != partition_name:
                in_names.append(name)
        elif alloc.kind == "ExternalOutput":
            out_names.append(name)
            out_avals.append(jax.core.ShapedArray(
                tuple(alloc.tensor_shape), _mybir.dt.np(alloc.dtype)))
    n_params = len(in_names)
    zero_out_shapes = [(a.shape, a.dtype) for a in out_avals]
    all_in_names = list(in_names) + list(out_names)
    if partition_name is not None:
        all_in_names.append(partition_name)

    def _body(*args):
        operands = list(args)
        if partition_name is not None:
            operands.append(b2j.partition_id_tensor())
        outs = b2j._bass_exec_p.bind(
            *operands,
            out_avals=tuple(out_avals),
            in_names=tuple(all_in_names),
            out_names=tuple(out_names),
            lowering_input_output_aliases=(),
            sim_require_finite=True,
            sim_require_nnan=True,
            nc=nc,
        )
        return tuple(outs)

    devices = jax.devices()[:N_CORES]
    mesh = Mesh(np.asarray(devices), ("core",))
    n_outs = len(out_names)
    in_specs = (PartitionSpec("core"),) * (n_params + n_outs)
    out_specs = (PartitionSpec("core"),) * n_outs
    fn = jax.jit(shard_map(_body, mesh=mesh, in_specs=in_specs,
                           out_specs=out_specs, check_rep=False),
                 keep_unused=True)
    runner = {
        "fn": fn,
        "in_names": in_names,
        "out_names": out_names,
        "out_avals": out_avals,
        "zero_out_shapes": zero_out_shapes,
        "mesh": mesh,
    }
    _CACHE["runner"] = runner
    return runner


def _run_spmd(in_maps):
    """Execute on 8 cores, returning list of per-core output dicts."""
    r = _build_runner()
    n_cores = N_CORES
    concat_in = [
        np.concatenate([np.asarray(in_maps[c][name]) for c in range(n_cores)],
                       axis=0)
        for name in r["in_names"]
    ]
    if "zeros" not in r:
        r["zeros"] = [np.zeros((n_cores * s[0], *s[1:]), d)
                      for s, d in r["zero_out_shapes"]]
    out_arrs = r["fn"](*concat_in, *r["zeros"])
    return [
        {name: np.asarray(out_arrs[i]).reshape(n_cores, *r["out_avals"][i].shape)[c]
         for i, name in enumerate(r["out_names"])}
        for c in range(n_cores)
    ]


def _prep_core_inputs(x, Wkqv, bkqv, Wo):
    """Host-side shard/pack. Returns (in_maps, host_bias) for 8 cores."""
    xT = [np.ascontiguousarray(x[b].T).astype(NP_BF16) for b in range(B)]
    per_g = []
    for g in range(4):
        h0 = 4 * g
        wkq = np.empty((E, 512), np.float32)
        for p in range(2):
            a, b_ = h0 + 2 * p, h0 + 2 * p + 1
            wkq[:, 256 * p:256 * p + 64] = Wkqv[a][:, 0:64]
            wkq[:, 256 * p + 64:256 * p + 128] = Wkqv[b_][:, 0:64]
            wkq[:, 256 * p + 128:256 * p + 192] = Wkqv[a][:, 64:128]
            wkq[:, 256 * p + 192:256 * p + 256] = Wkqv[b_][:, 64:128]
        wv = np.concatenate([Wkqv[h0 + h][:, 128:192] for h in range(HPC)],
                            axis=1)
        wog = Wo[256 * g:256 * (g + 1), :]
        bkq_arr = np.empty((128, 4), np.float32)
        for p in range(2):
            a, b_ = h0 + 2 * p, h0 + 2 * p + 1
            bkq_arr[0:64, 2 * p] = bkqv[a][0:64]
            bkq_arr[64:128, 2 * p] = bkqv[b_][0:64]
            bkq_arr[0:64, 2 * p + 1] = bkqv[a][64:128]
            bkq_arr[64:128, 2 * p + 1] = bkqv[b_][64:128]
        per_g.append({
            "wkq": wkq.astype(NP_BF16),
            "wv": wv.astype(NP_BF16),
            "wo": wog.astype(NP_BF16),
            "bkq": bkq_arr,
        })
    in_maps = []
    for c in range(N_CORES):
        b, g = c // 4, c % 4
        m = dict(per_g[g])
        m["xT"] = xT[b]
        in_maps.append(m)
    bv = np.concatenate([bkqv[h][128:192] for h in range(H)])
    return in_maps, bv


def kernel(x, Wkqv, bkqv, Wo, bo):
    x = np.asarray(x, np.float32)
    Wkqv = np.asarray(Wkqv, np.float32)
    bkqv = np.asarray(bkqv, np.float32)
    Wo = np.asarray(Wo, np.float32)
    bo = np.asarray(bo, np.float32)

    in_maps, bv = _prep_core_inputs(x, Wkqv, bkqv, Wo)
    results = _run_spmd(in_maps)
    partials = np.stack([results[c]["out"] for c in range(N_CORES)])
    partials = partials.reshape(B, 4, S, E).sum(axis=1)
    base = bv @ Wo + bo
    return (partials + base[None, None, :]).astype(np.float32)


# revision 11
# speedup vs baseline: 1.0504x; 1.0504x over previous
"""Causal self-attention (B=2, S=2048, E=1024, H=16, DH=64) on 8 trn2 cores.

Sharding: core c -> (batch b = c//4, head-group g = c%4, heads 4g..4g+3).
Data parallel over batch, tensor parallel over heads, row-sharded Wo;
partial outputs summed on host.

Per-core device kernel (bf16 matmuls, fp32 accumulation):
  phase A: kqT = (x @ Wkq)^T via W-stationary matmuls on xT (+bias on DVE),
           v   =  x @ Wv  (natural layout, [128 sk, 4 heads, 64 dh])
  phase B, per head-pair segment (c, p), per 128-sk tile j:
    st:  scores^T as a 64x128 row-tile PAIR (head A rows 0:64 / head B rows
         64:128) -> two PSUM banks; the pair CO-ISSUES on the PE (measured
         Δstart ~2ns), so both heads cost one N=w stream.
    exp: head A on DVE (Schraudolph bf16 bit-trick), head B on ACT (exact),
         diag tiles masked by a triangular bf16 multiply per head.
    av/rs (deferred 2 tiles for software pipelining): 128x64 col-tile PAIRS:
         av_bank[0:64]  = vA^T P_A   (tile_position (0,0))   } co-issue
         av_bank[64:128]= vB^T P_B   (tile_position (0,64))  }
         rs_bank[0:64]  = ones^T P_A  -> rowsum(A) broadcast to 64 partitions
         rs_bank[64:128]= ones^T P_B  (co-issued pair)
         Attention streams 3*w per tile (st w + av w + rs w) with both heads
         packed, and the softmax denominator arrives pre-broadcast.
    tail (deferred 1 segment): rc = exp(-ln(rs)) on ACT ([128,512] ops - no
         1-partition ops, no gpsimd broadcast), saT = av * rc on DVE.
  phase C: out = saT^T @ Wo in 32 half-tiles [128,512] through the shared
         1-bank PSUM ring, interleaved with the last segment's tail and the
         NEXT rep's projections so the PE never idles across rep boundaries.

PSUM: poolA = 4x 1-bank ring (st pairs, projections, phase C halves),
      poolB = 4x 1-bank ring (av/rs, 2 segments in flight). 8 banks total,
      allocated once - no per-rep pool churn.

kqT/v/saT are double-buffered by rep parity so rep r+1's projections can
overlap rep r's attention/output tail without WAR stalls.
"""
import math

import numpy as np
import ml_dtypes

import concourse.bass as bass
import concourse.bacc as bacc
import concourse.tile as tile
from concourse import mybir
from concourse.masks import make_upper_triangular

BF16 = mybir.dt.bfloat16
F32 = mybir.dt.float32
I16 = mybir.dt.int16
NP_BF16 = ml_dtypes.bfloat16

B, S, E, H, DH = 2, 2048, 1024, 16, 64
N_CORES = 8
HPC = 4          # heads per core
SCH = 4          # number of 512-wide sq chunks
SKT = 16         # number of 128-wide sk tiles
ET = 8           # number of 128-wide e tiles

Exp = mybir.ActivationFunctionType.Exp
Ln = mybir.ActivationFunctionType.Ln
Copy = mybir.ActivationFunctionType.Copy
Mult = mybir.AluOpType.mult
Add = mybir.AluOpType.add

# Schraudolph bf16 exp(s/8): i16 = round(s * SCHRA_A + SCHRA_B), reinterpret
# bits as bf16.  a = 2^7 / (8*ln2); b = 127*2^7 - c with c calibrated to
# center the relative error (max ~ +/-3%, which softmax normalization and
# the AV averaging wash down to ~1e-2 end-to-end).
SCHRA_A = 128.0 / (8.0 * math.log(2.0))
SCHRA_B = 127.0 * 128.0 - 4.6


def build_nc(reps=1):
    nc = bacc.Bacc(None, target_bir_lowering=False)

    xT = nc.dram_tensor("xT", [E, S], BF16, kind="ExternalInput")
    wkq = nc.dram_tensor("wkq", [E, 512], BF16, kind="ExternalInput")
    wv = nc.dram_tensor("wv", [E, 256], BF16, kind="ExternalInput")
    wo = nc.dram_tensor("wo", [256, E], BF16, kind="ExternalInput")
    bkq = nc.dram_tensor("bkq", [128, 4], F32, kind="ExternalInput")
    out = nc.dram_tensor("out", [S, E], F32, kind="ExternalOutput")

    with tile.TileContext(nc) as tc:
        import contextlib
        with contextlib.ExitStack() as ctx:
            const = ctx.enter_context(tc.tile_pool(name="const", bufs=1))
            wpool = ctx.enter_context(tc.tile_pool(name="wpool", bufs=1))
            xpool = ctx.enter_context(tc.tile_pool(name="xpool", bufs=1))
            kqpool = ctx.enter_context(tc.tile_pool(name="kqpool", bufs=1))
            vpool = ctx.enter_context(tc.tile_pool(name="vpool", bufs=1))
            sapool = ctx.enter_context(tc.tile_pool(name="sapool", bufs=1))
            pt_pool = ctx.enter_context(tc.tile_pool(name="pt", bufs=5))
            small = ctx.enter_context(tc.tile_pool(name="small", bufs=3))
            ostage = ctx.enter_context(tc.tile_pool(name="ostage", bufs=4))
            poolA = ctx.enter_context(
                tc.tile_pool(name="pA", bufs=4, space="PSUM"))
            poolB = ctx.enter_context(
                tc.tile_pool(name="pB", bufs=4, space="PSUM"))

            bkq_sb = const.tile([128, 4], F32)
            nc.sync.dma_start(bkq_sb[:], bkq[:])
            triu2 = const.tile([128, 2, 128], BF16)
            make_upper_triangular(nc, triu2[:, 0, :], val=1.0, diag=True)
            make_upper_triangular(nc, triu2[:, 1, :], val=1.0, diag=True)
            ones64 = const.tile([128, 64], BF16, tag="ones64")
            nc.vector.memset(ones64[:], 1.0)

            # interleave weight/activation loads so the first kq matmuls can
            # start as soon as their e-tile arrives
            wkq_sb = wpool.tile([128, ET, 512], BF16)
            xT_sb = xpool.tile([128, ET, S], BF16)
            for e in range(ET):
                nc.sync.dma_start(wkq_sb[:, e, :], wkq[128 * e:128 * (e + 1), :])
                nc.sync.dma_start(xT_sb[:, e, :], xT[128 * e:128 * (e + 1), :])
            wv_sb = wpool.tile([128, ET, 256], BF16)
            nc.sync.dma_start(wv_sb[:], wv.rearrange("(n p) f -> p n f", p=128))
            wo_sb = wpool.tile([128, 2, E], BF16)
            nc.sync.dma_start(wo_sb[:], wo.rearrange("(n p) f -> p n f", p=128))

            # double-buffered by rep parity (cross-rep WAR decoupling)
            kqT_db = [kqpool.tile([128, 4, S], BF16, tag=f"kqT{i}",
                                  name=f"kqT{i}") for i in range(2)]
            v_db = [vpool.tile([128, SKT, HPC, DH], BF16, tag=f"v{i}",
                               name=f"v{i}") for i in range(2)]
            saT_db = [sapool.tile([128, 2, S], BF16, tag=f"saT{i}",
                                  name=f"saT{i}") for i in range(2)]

            def psA():
                return poolA.tile([128, 512], F32, tag="w1", name="w1")

            def psB():
                return poolB.tile([128, 512], F32, tag="w2", name="w2")

            _prev_sa = [None]
            for _rep in range(reps):
                kqT_sb = kqT_db[_rep % 2]
                v_sb = v_db[_rep % 2]
                saT_sb = saT_db[_rep % 2]

                def kq_proj(blk, c):
                    ps = psA()
                    for e in range(ET):
                        nc.tensor.matmul(
                            ps[:], wkq_sb[:, e, 128 * blk:128 * (blk + 1)],
                            xT_sb[:, e, 512 * c:512 * (c + 1)],
                            start=(e == 0), stop=(e == ET - 1))
                    nc.vector.tensor_scalar_add(
                        kqT_sb[:, blk, 512 * c:512 * (c + 1)], ps[:],
                        bkq_sb[:, blk:blk + 1])

                def v_proj(t0, t1):
                    # two s-tiles share one PSUM bank; one [128, 512] eviction
                    for t in range(t0, t1, 2):
                        ps = psA()
                        for half in (0, 1):
                            for e in range(ET):
                                nc.tensor.matmul(
                                    ps[:, 256 * half:256 * (half + 1)],
                                    xT_sb[:, e,
                                          128 * (t + half):128 * (t + half + 1)],
                                    wv_sb[:, e, :],
                                    start=(e == 0), stop=(e == ET - 1))
                        nc.scalar.copy(
                            v_sb[:, t:t + 2, :, :],
                            ps[:].rearrange("p (t h d) -> p t h d", t=2, h=HPC))

                av_q = []

                def flush_av(n=0):
                    # Emit deferred av/rs groups (software pipelining: the
                    # PE queue is strict FIFO, so av(j) - which waits on
                    # exp(j) - must queue BEHIND st(j+1)/st(j+2) or the PE
                    # head-blocks). Each group is 4 matmuls in 128x64 mode:
                    # av col-pair + rowsum col-pair, each pair co-issuing.
                    while len(av_q) > n:
                        avb, rsb, pt, p, j, off, first, last = av_q.pop(0)
                        nc.tensor.matmul(avb[0:64, off:512],
                                         v_sb[:, j, 2 * p, :],
                                         pt[:, 0, off:512], start=first,
                                         stop=last, tile_position=(0, 0))
                        nc.tensor.matmul(avb[64:128, off:512],
                                         v_sb[:, j, 2 * p + 1, :],
                                         pt[:, 1, off:512], start=first,
                                         stop=last, tile_position=(0, 64))
                        nc.tensor.matmul(rsb[0:64, off:512], ones64[:],
                                         pt[:, 0, off:512], start=first,
                                         stop=last, tile_position=(0, 0))
                        nc.tensor.matmul(rsb[64:128, off:512], ones64[:],
                                         pt[:, 1, off:512], start=first,
                                         stop=last, tile_position=(0, 64))

                def attn_tiles(c, p):
                    """One head-pair segment: st pair per tile (64x128 mode,
                    co-issued), exp on DVE+ACT, av/rs groups deferred 2 tiles
                    to batch mode switches and hide exp latency."""
                    sq0 = 512 * c
                    kblk, qblk = 2 * p, 2 * p + 1
                    avb, rsb = psB(), psB()
                    nj = 4 * c + 4
                    for j in range(nj):
                        r = j - 4 * c
                        diag = r >= 0
                        off = 128 * r if diag else 0
                        w = 512 - off
                        st0, st1 = psA(), psA()
                        nc.tensor.matmul(
                            st0[:, 0:w],
                            kqT_sb[0:64, kblk, 128 * j:128 * (j + 1)],
                            kqT_sb[0:64, qblk, sq0 + off:sq0 + 512],
                            start=True, stop=True, tile_position=(0, 0))
                        nc.tensor.matmul(
                            st1[:, 0:w],
                            kqT_sb[64:128, kblk, 128 * j:128 * (j + 1)],
                            kqT_sb[64:128, qblk, sq0 + off:sq0 + 512],
                            start=True, stop=True, tile_position=(64, 0))
                        pt = pt_pool.tile([128, 2, 512], BF16, tag="pt")
                        nc.vector.tensor_scalar(
                            pt[:, 0, off:512].bitcast(I16),
                            st0[:, 0:w], SCHRA_A, SCHRA_B, Mult, Add)
                        nc.scalar.activation(pt[:, 1, off:512], st1[:, 0:w],
                                             Exp, scale=0.125)
                        if diag:
                            nc.vector.tensor_mul(
                                pt[:, :, off:off + 128],
                                pt[:, :, off:off + 128], triu2[:])
                        av_q.append((avb, rsb, pt, p, j, off, j == 0,
                                     j == nj - 1))
                        flush_av(3)
                    return avb, rsb

                def attn_tail(c, p, avb, rsb):
                    # rc = exp(-ln(rowsum)) on ACT, full-width [128, 512]
                    # (rowsums arrive pre-broadcast from the rs matmuls);
                    # saT = av * rc fused in the PSUM eviction on DVE.
                    sq0 = 512 * c
                    lt = small.tile([128, 512], F32, tag="lt")
                    nc.scalar.activation(lt[:], rsb[:], Ln)
                    rc = small.tile([128, 512], F32, tag="rc")
                    nc.scalar.activation(rc[:], lt[:], Exp, scale=-1.0)
                    nc.vector.tensor_mul(saT_sb[:, p, sq0:sq0 + 512],
                                         avb[:], rc[:])

                def phase_c(sa, t0, t1):
                    # out = sa^T @ Wo in [128, 512] half-tiles through poolA
                    for t in range(t0, t1):
                        for n in range(2):
                            ps = psA()
                            nc.tensor.matmul(
                                ps[:], sa[:, 0, 128 * t:128 * (t + 1)],
                                wo_sb[:, 0, 512 * n:512 * (n + 1)],
                                start=True, stop=False)
                            nc.tensor.matmul(
                                ps[:], sa[:, 1, 128 * t:128 * (t + 1)],
                                wo_sb[:, 1, 512 * n:512 * (n + 1)],
                                start=False, stop=True)
                            ot = ostage.tile([128, 512], F32, tag="ot")
                            if (2 * t + n) % 2 == 0:
                                nc.vector.tensor_copy(ot[:], ps[:])
                            else:
                                nc.scalar.copy(ot[:], ps[:])
                            nc.sync.dma_start(
                                out[128 * t:128 * (t + 1),
                                    512 * n:512 * (n + 1)], ot[:])

                # upfront: only what segment (0,*) needs; the rest of the
                # projections are emitted between segments as PE filler
                for blk in range(4):
                    kq_proj(blk, 0)
                v_proj(0, 4)
                # the previous rep's last phase C chunk (t=12..15 waits on
                # that rep's final tail) is deferred to HERE so its wait
                # hides under this rep's upfront projections
                if _prev_sa[0] is not None:
                    phase_c(_prev_sa[0], 12, 16)
                fillers = {
                    2: lambda: (kq_proj(0, 1), kq_proj(1, 1), v_proj(4, 8)),
                    3: lambda: (kq_proj(2, 1), kq_proj(3, 1)),
                    4: lambda: (kq_proj(0, 2), kq_proj(1, 2), v_proj(8, 12)),
                    5: lambda: (kq_proj(2, 2), kq_proj(3, 2)),
                    6: lambda: (kq_proj(0, 3), kq_proj(1, 3), v_proj(12, 16)),
                    7: lambda: (kq_proj(2, 3), kq_proj(3, 3)),
                }
                pending = None
                for s, (c, p) in enumerate((c, p) for c in range(SCH)
                                           for p in (0, 1)):
                    if s in fillers:
                        fillers[s]()
                    avb, rsb = attn_tiles(c, p)
                    if pending is not None:
                        attn_tail(*pending)
                    pending = (c, p, avb, rsb)
                flush_av(0)
                # phase C interleaved with the last segment's tail: t<12 only
                # needs chunks c<3, whose tails are long done.
                phase_c(saT_sb, 0, 8)
                attn_tail(*pending)
                phase_c(saT_sb, 8, 12)
                _prev_sa[0] = saT_sb
            phase_c_last = _prev_sa[0]
            if phase_c_last is not None:
                phase_c(phase_c_last, 12, 16)

    nc.compile()
    _merge_act_table_loads(nc)
    return nc


def _merge_act_table_loads(nc):
    """Retarget all ACT table loads to natural_log_exp_and_others (holds both
    Exp and Ln) and drop the now-redundant reloads - the default chooser
    alternates exp_and_others/natural_log, reloading tables (~2.7us each)
    at every Exp<->Ln switch."""
    from concourse.hw_specs import get_activation_tables
    tables = get_activation_tables(nc.m.arch)
    combined = list(tables).index("natural_log_exp_and_others")
    fns = tables["natural_log_exp_and_others"]
    for f in nc.m.functions:
        for b in f.blocks:
            first = True
            drop = []
            for ii, ins in enumerate(b.instructions):
                if isinstance(ins, mybir.InstLoadActFuncSet):
                    if first:
                        ins.act_func_set_id = combined
                        first = False
                    else:
                        assert ins.sync_info is None
                        drop.append(ii)
                elif isinstance(ins, mybir.InstActivation):
                    assert ins.func in fns or ins.func in (
                        mybir.ActivationFunctionType.Copy,
                        mybir.ActivationFunctionType.Identity), ins.func
            for ii in reversed(drop):
                del b.instructions[ii]


_CACHE = {}


def _build_runner():
    """Build the SPMD PJRT executable once; returns a dict with a jitted fn.

    Mirrors concourse.bass2jax.run_bass_via_pjrt but hoisted so repeated
    kernel() calls reuse the traced/compiled executable. No donation: the
    kernel DMA-writes every output element, so uninitialized output buffers
    are fine.
    """
    import jax
    from jax.sharding import Mesh, PartitionSpec
    from jax.experimental.shard_map import shard_map
    from concourse import bass2jax as b2j
    from concourse import mybir as _mybir

    if "runner" in _CACHE:
        return _CACHE["runner"]

    nc = _CACHE.get("nc")
    if nc is None:
        nc = _CACHE["nc"] = build_nc()

    b2j.install_neuronx_cc_hook()
    partition_name = (nc.partition_id_tensor.name
                      if nc.partition_id_tensor else None)

    in_names, out_names, out_avals = [], [], []
    for alloc in nc.m.functions[0].allocations:
        if not isinstance(alloc, _mybir.MemoryLocationSet):
            continue
        name = alloc.memorylocations[0].name
        if alloc.kind == "ExternalInput":
            if name != partition_name:
                in_names.append(name)
        elif alloc.kind == "ExternalOutput":
            out_names.append(name)
            out_avals.append(jax.core.ShapedArray(
                tuple(alloc.tensor_shape), _mybir.dt.np(alloc.dtype)))
    n_params = len(in_names)
    zero_out_shapes = [(a.shape, a.dtype) for a in out_avals]
    all_in_names = list(in_names) + list(out_names)
    if partition_name is not None:
        all_in_names.append(partition_name)

    def _body(*args):
        operands = list(args)
        if partition_name is not None:
            operands.append(b2j.partition_id_tensor())
        outs = b2j._bass_exec_p.bind(
            *operands,
            out_avals=tuple(out_avals),
            in_names=tuple(all_in_names),
            out_names=tuple(out_names),
            lowering_input_output_aliases=(),
            sim_require_finite=True,
            sim_require_nnan=True,
            nc=nc,
        )
        return tuple(outs)

    devices = jax.devices()[:N_CORES]
    mesh = Mesh(np.asarray(devices), ("core",))
    n_outs = len(out_names)
    in_specs = (PartitionSpec("core"),) * (n_params + n_outs)
    out_specs = (PartitionSpec("core"),) * n_outs
    fn = jax.jit(shard_map(_body, mesh=mesh, in_specs=in_specs,
                           out_specs=out_specs, check_rep=False),
                 keep_unused=True)
    runner = {
        "fn": fn,
        "in_names": in_names,
        "out_names": out_names,
        "out_avals": out_avals,
        "zero_out_shapes": zero_out_shapes,
        "mesh": mesh,
    }
    _CACHE["runner"] = runner
    return runner


def _run_spmd(in_maps):
    """Execute on 8 cores, returning list of per-core output dicts."""
    r = _build_runner()
    n_cores = N_CORES
    concat_in = [
        np.concatenate([np.asarray(in_maps[c][name]) for c in range(n_cores)],
                       axis=0)
        for name in r["in_names"]
    ]
    if "zeros" not in r:
        r["zeros"] = [np.zeros((n_cores * s[0], *s[1:]), d)
                      for s, d in r["zero_out_shapes"]]
    out_arrs = r["fn"](*concat_in, *r["zeros"])
    return [
        {name: np.asarray(out_arrs[i]).reshape(n_cores, *r["out_avals"][i].shape)[c]
         for i, name in enumerate(r["out_names"])}
        for c in range(n_cores)
    ]


def _prep_core_inputs(x, Wkqv, bkqv, Wo):
    """Host-side shard/pack. Returns (in_maps, host_bias) for 8 cores."""
    xT = [np.ascontiguousarray(x[b].T).astype(NP_BF16) for b in range(B)]
    per_g = []
    for g in range(4):
        h0 = 4 * g
        wkq = np.empty((E, 512), np.float32)
        for p in range(2):
            a, b_ = h0 + 2 * p, h0 + 2 * p + 1
            wkq[:, 256 * p:256 * p + 64] = Wkqv[a][:, 0:64]
            wkq[:, 256 * p + 64:256 * p + 128] = Wkqv[b_][:, 0:64]
            wkq[:, 256 * p + 128:256 * p + 192] = Wkqv[a][:, 64:128]
            wkq[:, 256 * p + 192:256 * p + 256] = Wkqv[b_][:, 64:128]
        wv = np.concatenate([Wkqv[h0 + h][:, 128:192] for h in range(HPC)],
                            axis=1)
        wog = Wo[256 * g:256 * (g + 1), :]
        bkq_arr = np.empty((128, 4), np.float32)
        for p in range(2):
            a, b_ = h0 + 2 * p, h0 + 2 * p + 1
            bkq_arr[0:64, 2 * p] = bkqv[a][0:64]
            bkq_arr[64:128, 2 * p] = bkqv[b_][0:64]
            bkq_arr[0:64, 2 * p + 1] = bkqv[a][64:128]
            bkq_arr[64:128, 2 * p + 1] = bkqv[b_][64:128]
        per_g.append({
            "wkq": wkq.astype(NP_BF16),
            "wv": wv.astype(NP_BF16),
            "wo": wog.astype(NP_BF16),
            "bkq": bkq_arr,
        })
    in_maps = []
    for c in range(N_CORES):
        b, g = c // 4, c % 4
        m = dict(per_g[g])
        m["xT"] = xT[b]
        in_maps.append(m)
    bv = np.concatenate([bkqv[h][128:192] for h in range(H)])
    return in_maps, bv


def kernel(x, Wkqv, bkqv, Wo, bo):
    x = np.asarray(x, np.float32)
    Wkqv = np.asarray(Wkqv, np.float32)
    bkqv = np.asarray(bkqv, np.float32)
    Wo = np.asarray(Wo, np.float32)
    bo = np.asarray(bo, np.float32)

    in_maps, bv = _prep_core_inputs(x, Wkqv, bkqv, Wo)
    results = _run_spmd(in_maps)
    partials = np.stack([results[c]["out"] for c in range(N_CORES)])
    partials = partials.reshape(B, 4, S, E).sum(axis=1)
    base = bv @ Wo + bo
    return (partials + base[None, None, :]).astype(np.float32)
